# revision 48
# baseline (speedup 1.0000x reference)
"""MLA attention (DeepSeek-style) distributed over 8 TRN2 NeuronCores.

Sharding: core c -> batch b=c//4, head-group/seq-shard g=c%4.
Phase A: down-proj of own 512-pos shard (slab-contiguous weight layout
for 4KB/partition DMA runs; PE warmup matmuls cover the p-state ramp;
a tiny dummy collective fired first absorbs cross-core start skew) ->
bf16 bounce -> 8-core AllGathers (kv 577 rows, then q in two sub-1MB
pieces; each core reads only its batch's half via cfg base regs).
Phase B: up-proj. kt from UNSCALED latent with inv-rms applied at
eviction (so kt only waits on the kv AG, not the stat row); kv_lat
prescaled in place before v; chunks interleaved so DVE prescale hides
under PE. Single 128-wide rope chain covers both heads of a qt_pe pair.
Queue discipline: gather loads are ordered by the AG that gates them
(akv/aq rows ride the gpsimd queue, which blocks on exactly those
collectives).
Attention: flash-style causal, St[kv,q] layout. Causal mask PREFILLED
into PSUM by the DVE so score matmuls (start=False) accumulate onto it
and exp depends only on the matmul stop; softmax denominators via
ones-matmul; PV/lt flushed two blocks behind scores; per-qc eviction
(vector reciprocal + gpsimd broadcast). Outputs leave via a per-head
8-core AllToAll (1MB instead of an AllGather's 4.2MB write) whose
received row block (b*4+k) is exactly the wo rhs; the row-parallel wo
matmul for head h-1 is interleaved behind head h's attention and
accumulated in SBUF; bf16 output stores drain on two DMA queues.
"""

import numpy as np
import ml_dtypes

import concourse.bass as bass
import concourse.bacc as bacc
import concourse.tile as tile
import concourse.mybir as mybir
from concourse.bass_utils import run_bass_kernel_spmd

BF16 = ml_dtypes.bfloat16

# problem constants (hardcoded per harness rules)
DIM = 2048
N_HEADS = 16
Q_LORA = 1536
KV_LORA = 512
NOPE = 128
ROPE = 64
V_DIM = 128
QK_HD = NOPE + ROPE  # 192
EPS = 1e-6
B, S = 2, 2048
SCALE = QK_HD ** -0.5

NCORES = 8
GROUP = 4               # cores per batch
SSH = S // GROUP        # 512, seq shard
HPC = N_HEADS // GROUP  # 4 heads per core
P = 128
NKT = DIM // P          # 16
NQM = Q_LORA // P       # 12
NQ1 = 7                 # q slabs in first AG
NKVM = KV_LORA // P     # 4
NCH = S // 512          # 4
KVROWS = KV_LORA + ROPE + 1   # 577 kv bounce rows
Q1ROWS = NQ1 * P + 1          # 897 (incl a_q row; 918KB < 1MB AG limit)
Q2ROWS = (NQM - NQ1) * P      # 640

_cache = {}


def _build():
    nc = bacc.Bacc("TRN2", target_bir_lowering=False, debug=False,
                   num_devices=NCORES)
    f32 = mybir.dt.float32
    bf = mybir.dt.bfloat16

    # ---- dram parameters (slab-contiguous layouts; see _prep_inputs) ----
    xT = nc.dram_tensor("xT", [DIM, SSH], bf, kind="ExternalInput")
    wqa_sl = nc.dram_tensor("wqa_sl", [NQM * P, NKT * P], bf,
                            kind="ExternalInput")
    wkva_sl = nc.dram_tensor("wkva_sl", [NKVM * P, NKT * P], bf,
                             kind="ExternalInput")
    wkvr_sl = nc.dram_tensor("wkvr_sl", [P, NKT * ROPE], bf,
                             kind="ExternalInput")
    wqbT = nc.dram_tensor("wqbT", [Q_LORA, HPC * QK_HD], bf,
                          kind="ExternalInput")
    wkvbT = nc.dram_tensor("wkvbT", [KV_LORA, HPC * (NOPE + V_DIM)], bf,
                           kind="ExternalInput")
    # wo.T rows regrouped head-major: rows [h*512 + j*128 ...] = head j*4+h
    woTr = nc.dram_tensor("woTr", [N_HEADS * V_DIM, DIM], bf,
                          kind="ExternalInput")
    cos_sh = nc.dram_tensor("cos_sh", [P, SSH], bf, kind="ExternalInput")
    sin_sh = nc.dram_tensor("sin_sh", [P, SSH], bf, kind="ExternalInput")
    cos_full = nc.dram_tensor("cos_full", [P, S], bf, kind="ExternalInput")
    sin_full = nc.dram_tensor("sin_full", [P, S], bf, kind="ExternalInput")
    perm64 = nc.dram_tensor("perm64", [P, P], bf, kind="ExternalInput")
    trimask = nc.dram_tensor("trimask", [P, 512], f32, kind="ExternalInput")
    outT = nc.dram_tensor("out", [DIM, SSH], bf, kind="ExternalOutput")

    cfg = nc.dram_tensor("cfg", [1, 5], mybir.dt.int32, kind="ExternalInput")

    # ---- internal dram (collective bounce buffers) ----
    # 8-core AllGathers (shared-out mesh path needs >4 cores/group); each
    # core reads only its own batch's half via the cfg base offsets.
    b_kv = nc.dram_tensor("b_kv", [KVROWS, SSH], bf)
    g_kv = nc.dram_tensor("g_kv", [NCORES * KVROWS, SSH], bf,
                          addr_space="Shared")
    b_q1 = nc.dram_tensor("b_q1", [Q1ROWS, SSH], bf)
    g_q1 = nc.dram_tensor("g_q1", [NCORES * Q1ROWS, SSH], bf,
                          addr_space="Shared")
    b_q2 = nc.dram_tensor("b_q2", [Q2ROWS, SSH], bf)
    g_q2 = nc.dram_tensor("g_q2", [NCORES * Q2ROWS, SSH], bf,
                          addr_space="Shared")
    # per-head output AllToAll: shard (b*4+qc) -> rank (b, qc); the other
    # batch's shards carry garbage their recipients never read
    b_o = [nc.dram_tensor(f"b_o{h}", [NCORES * V_DIM, SSH], bf)
           for h in range(HPC)]
    o_t = [nc.dram_tensor(f"o_t{h}", [NCORES * V_DIM, SSH], bf)
           for h in range(HPC)]
    b_sk = nc.dram_tensor("b_sk", [1, 64], bf)
    g_sk = nc.dram_tensor("g_sk", [NCORES, 64], bf, addr_space="Shared")
    rg8 = [list(range(NCORES))]
    rg4 = [[0, 1, 2, 3], [4, 5, 6, 7]]

    with tile.TileContext(nc) as tc:
        with (
            tc.tile_pool(name="persist", bufs=1) as persist,
            tc.tile_pool(name="attn", bufs=1) as attn_pool,
            tc.tile_pool(name="wts", bufs=1) as wts,
        ):
            # constants (cheap, engine-local)
            ones_f = persist.tile([P, 1], f32)
            nc.vector.memset(ones_f, 1.0)
            ones_b = persist.tile([P, 1], bf)
            nc.vector.memset(ones_b, 1.0)
            eps_sb = persist.tile([1, 1], f32)
            nc.vector.memset(eps_sb, EPS)
            warm_w = persist.tile([P, P], bf)
            nc.vector.memset(warm_w, 0.0)
            warm_x = persist.tile([P, 512], bf)
            nc.vector.memset(warm_x, 0.0)
            cfg_sb = persist.tile([1, 5], mybir.dt.int32)
            nc.sync.dma_start(out=cfg_sb, in_=cfg[:])

            # per-core dynamic base offsets (own batch's half of gathers)
            def snap_cfg(i, maxv):
                r = nc.alloc_registers()
                nc.regs_load(r, cfg_sb[0:1, i:i + 1])
                return nc.snap(r, donate=True, min_val=0, max_val=maxv)

            kv_base = snap_cfg(0, GROUP * KVROWS)
            q1_base = snap_cfg(1, GROUP * Q1ROWS)
            q2_base = snap_cfg(2, GROUP * Q2ROWS)
            ob_row = snap_cfg(3, GROUP * V_DIM)

            # attention-phase persistent tiles (filled by up-proj)
            qt_nope = [attn_pool.tile([P, S], bf, tag=f"qtn{h}",
                                      name=f"qt_nope{h}") for h in range(HPC)]
            qt_pe = [attn_pool.tile([P, S], bf, tag=f"qtp{h}",
                                    name=f"qt_pe{h}")
                     for h in range(HPC // 2)]
            kt_nope = [attn_pool.tile([P, S], bf, tag=f"ktn{h}",
                                      name=f"kt_nope{h}") for h in range(HPC)]
            v_all = attn_pool.tile([P, S // P, HPC * V_DIM], bf)
            kpe_dup = attn_pool.tile([P, NCH, 512], bf)

            # tiny dummy collective fired first: absorbs the cross-core
            # start skew during phase-A compute, so the first real AG's
            # all-ready gate passes immediately
            sk = persist.tile([1, 64], bf)
            nc.vector.memset(sk, 0.0)
            nc.scalar.dma_start(out=b_sk[:], in_=sk)
            nc.gpsimd.collective_compute(
                "AllGather", mybir.AluOpType.bypass, replica_groups=rg8,
                ins=[b_sk[:]], outs=[g_sk[:]])

            # weights/tables prefetched on the scalar HWDGE queue (sync
            # queue is reserved for x + down-proj slabs early on)
            perm_sb = persist.tile([P, P], bf)
            nc.scalar.dma_start(out=perm_sb, in_=perm64[:])
            cos_sh_sb = persist.tile([P, SSH], bf)
            nc.scalar.dma_start(out=cos_sh_sb, in_=cos_sh[:])
            sin_sh_sb = persist.tile([P, SSH], bf)
            nc.scalar.dma_start(out=sin_sh_sb, in_=sin_sh[:])
            wkvb = wts.tile([P, NKVM, HPC * (NOPE + V_DIM)], bf)
            wqb = wts.tile([P, NQM, HPC * QK_HD], bf)
            mask_sb = persist.tile([P, 512], f32)
            cos_f_sb = persist.tile([P, S], bf)
            sin_f_sb = persist.tile([P, S], bf)

            # ======== Phase A + up-proj (shared latent pool) ========
            up_lat_cm = tc.tile_pool(name="up_lat", bufs=1)
            up_lat = up_lat_cm.__enter__()
            with (
                tc.tile_pool(name="pa", bufs=2) as pa,
                tc.tile_pool(name="pa_x", bufs=1) as pa_x,
                tc.tile_pool(name="pa_out", bufs=3) as pa_out,
                tc.tile_pool(name="pa_ps", bufs=2, space="PSUM") as pa_ps,
                tc.tile_pool(name="pa_st", bufs=1, space="PSUM") as pa_st,
                tc.tile_pool(name="warm", bufs=1, space="PSUM") as warm_pool,
            ):
                # PE p-state warmup: keep the array streaming from t=0 so
                # the first real matmuls run at full clock
                warm_ps = warm_pool.tile([P, 512], f32)
                for _ in range(12):
                    nc.tensor.matmul(warm_ps, warm_w, warm_x,
                                     start=True, stop=True)

                # first x chunk + first slab lead the sync queue so the
                # first real matmul fires ~4us in; rest follow
                x_all = pa_x.tile([P, NKT, SSH], bf)
                nc.sync.dma_start(
                    out=x_all[:, 0:4, :],
                    in_=xT[0:4 * P, :].rearrange("(kt p) s -> p kt s", p=P))
                slab0 = pa.tile([P, NKT * P], bf, tag="slab")
                nc.sync.dma_start(out=slab0, in_=wkva_sl[0:P, :])
                for xq in range(1, 4):
                    nc.scalar.dma_start(
                        out=x_all[:, xq * 4:(xq + 1) * 4, :],
                        in_=xT[xq * 4 * P:(xq + 1) * 4 * P, :].rearrange(
                            "(kt p) s -> p kt s", p=P))

                q_stat = pa_st.tile([1, SSH], f32)
                kv_stat = pa_st.tile([1, SSH], f32)
                pending_stat = []

                def flush_stat():
                    while pending_stat:
                        ps, mrows, stat_ps, first, last = pending_stat.pop(0)
                        sq = pa.tile([P, SSH], f32, tag="sq")
                        nc.scalar.square(sq[:mrows, :], ps[:mrows, :])
                        nc.tensor.matmul(stat_ps, ones_f[:mrows, :],
                                         sq[:mrows, :], start=first,
                                         stop=last)

                def down_slab(wsl, row0, mrows, mw, bounce, dst_rows,
                              stat_ps, stat_first, stat_last, ev_tag="ev",
                              slab=None):
                    # wsl rows row0:row0+128 hold a slab laid out
                    # [p, kt*mw + m] (contiguous 4KB/partition DMA)
                    if slab is None:
                        slab = pa.tile([P, NKT * mw], bf, tag="slab")
                        nc.sync.dma_start(out=slab, in_=wsl[row0:row0 + P, :])
                    ps = pa_ps.tile([P, SSH], f32, tag="dps")
                    for k in range(NKT):
                        nc.tensor.matmul(ps[:mrows, :],
                                         slab[:, k * mw:k * mw + mrows],
                                         x_all[:, k, :], start=(k == 0),
                                         stop=(k == NKT - 1))
                    ev = pa_out.tile([P, SSH], bf, tag=ev_tag)
                    nc.vector.tensor_copy(ev[:mrows, :], ps[:mrows, :])
                    if dst_rows is not None:
                        nc.scalar.dma_start(
                            out=bounce[dst_rows:dst_rows + mrows, :],
                            in_=ev[:mrows, :])
                    if stat_ps is not None:
                        pending_stat.append((ps, mrows, stat_ps, stat_first,
                                             stat_last))
                    return ev

                def stat_row(stat, n, bounce, row):
                    tmp = pa.tile([1, SSH], f32, tag="srt")
                    nc.scalar.activation(tmp, stat,
                                         mybir.ActivationFunctionType.Sqrt,
                                         bias=eps_sb[0:1, 0:1], scale=1.0 / n)
                    rcp = pa.tile([1, SSH], f32, tag="rcp")
                    nc.vector.reciprocal(rcp, tmp)
                    rb = pa.tile([1, SSH], bf, tag="rb")
                    nc.vector.tensor_copy(rb, rcp)
                    nc.scalar.dma_start(out=bounce[row:row + 1, :], in_=rb)

                # ---- kv first (single 577-row AG; the all-ready gate is
                # start-skew bound, so splitting buys nothing)
                for m in range(NKVM):
                    down_slab(wkva_sl, m * P, P, P, b_kv, m * P, kv_stat,
                              m == 0, m == NKVM - 1,
                              slab=slab0 if m == 0 else None)
                kpe_ev = down_slab(wkvr_sl, 0, ROPE, ROPE, None, None, None,
                                   False, False, ev_tag="kpe_ev")
                xs_ps = pa_ps.tile([ROPE, SSH], f32, tag="xs")
                nc.tensor.matmul(xs_ps, perm_sb[:ROPE, :ROPE], kpe_ev[:ROPE, :])
                y0 = pa.tile([ROPE, SSH], bf, tag="ry0")
                nc.vector.tensor_mul(y0, kpe_ev[:ROPE, :], cos_sh_sb[:ROPE, :])
                y1 = pa.tile([ROPE, SSH], bf, tag="ry1")
                nc.vector.tensor_mul(y1, xs_ps, sin_sh_sb[:ROPE, :])
                yr = pa.tile([ROPE, SSH], bf, tag="ryr")
                nc.vector.tensor_add(yr, y0, y1)
                nc.scalar.dma_start(out=b_kv[KV_LORA:KV_LORA + ROPE, :],
                                    in_=yr)
                flush_stat()
                stat_row(kv_stat, KV_LORA, b_kv, KVROWS - 1)
                nc.gpsimd.collective_compute(
                    "AllGather", mybir.AluOpType.bypass, replica_groups=rg8,
                    ins=[b_kv[:]], outs=[g_kv[:]])

                # weight/table prefetch on scalar HWDGE (needed from the
                # up-proj on)
                nc.scalar.dma_start(
                    out=wkvb,
                    in_=wkvbT[:].rearrange("(kt p) m -> p kt m", p=P))
                nc.scalar.dma_start(
                    out=wqb, in_=wqbT[:].rearrange("(kt p) m -> p kt m", p=P))
                nc.scalar.dma_start(out=mask_sb, in_=trimask[:])
                nc.scalar.dma_start(out=cos_f_sb, in_=cos_full[:])
                nc.scalar.dma_start(out=sin_f_sb, in_=sin_full[:])

                # ---- q down-proj (overlaps AG_kv) ----
                for m in range(NQ1):
                    down_slab(wqa_sl, m * P, P, P, b_q1, m * P, q_stat,
                              m == 0, False)

                # kv gather loads. Queue discipline: the AG1-gated kv_lat
                # load goes FIRST on sync (one big DMA); AG2-gated loads
                # after it; AG2/AG4-gated row loads ride the gpsimd queue,
                # which is blocked on exactly those collectives anyway.
                kv_lat = up_lat.tile([P, NCH, NKVM, 512], bf)
                for r in range(NCH):
                    nc.sync.dma_start(
                        out=kv_lat[:, r, :, :],
                        in_=g_kv[bass.ds(kv_base + r * KVROWS, KV_LORA), :]
                        .rearrange("(kt p) s -> p kt s", p=P))
                akv_row = up_lat.tile([1, NCH, 512], bf)
                for r in range(NCH):
                    nc.gpsimd.dma_start(
                        out=akv_row[0:1, r, :],
                        in_=g_kv[bass.ds(kv_base + r * KVROWS + KVROWS - 1, 1), :])
                a_kv_bc = up_lat.tile([P, NCH, 512], bf)
                for r in range(NCH):
                    nc.gpsimd.partition_broadcast(a_kv_bc[:, r, :],
                                                  akv_row[0:1, r, :])
                for r in range(NCH):
                    nc.sync.dma_start(
                        out=kpe_dup[:ROPE, r, :],
                        in_=g_kv[bass.ds(kv_base + r * KVROWS + KV_LORA, ROPE), :])
                    nc.sync.dma_start(
                        out=kpe_dup[ROPE:, r, :],
                        in_=g_kv[bass.ds(kv_base + r * KVROWS + KV_LORA, ROPE), :])

                for m in range(NQ1, NQM):
                    down_slab(wqa_sl, m * P, P, P, b_q2, (m - NQ1) * P,
                              q_stat, False, m == NQM - 1)
                flush_stat()
                stat_row(q_stat, Q_LORA, b_q1, Q1ROWS - 1)
                nc.gpsimd.collective_compute(
                    "AllGather", mybir.AluOpType.bypass, replica_groups=rg8,
                    ins=[b_q1[:]], outs=[g_q1[:]])

                # a_q rows ride AG_q1, so their loads/broadcasts slot in
                # on the gpsimd queue BETWEEN the two q-AG waits; the qt
                # evictions then unblock ~30us before AG_q2 lands
                aq_row = up_lat.tile([1, NCH, 512], bf)
                for r in range(NCH):
                    nc.gpsimd.dma_start(
                        out=aq_row[0:1, r, :],
                        in_=g_q1[bass.ds(q1_base + r * Q1ROWS + Q1ROWS - 1,
                                         1), :])
                a_q_bc = up_lat.tile([P, NCH, 512], bf)
                for r in range(NCH):
                    nc.gpsimd.partition_broadcast(a_q_bc[:, r, :],
                                                  aq_row[0:1, r, :])
                nc.gpsimd.collective_compute(
                    "AllGather", mybir.AluOpType.bypass, replica_groups=rg8,
                    ins=[b_q2[:]], outs=[g_q2[:]])

            # ================= Phase B: up projections =================
            with (
                tc.tile_pool(name="up", bufs=3) as up,
                tc.tile_pool(name="qlat", bufs=2) as qlat_pool,
                tc.tile_pool(name="up_ps", bufs=4, space="PSUM") as up_ps,
                tc.tile_pool(name="pe_ps", bufs=2, space="PSUM") as pe_ps,
            ):
                # k_nope from UNSCALED latent (inv-rms applied at eviction,
                # so these matmuls only wait on AG_kvlat); v from latent
                # prescaled in place. Interleave chunks so the DVE prescale
                # of chunk c runs while the PE does kt of chunk c+1.
                def kt_up(c):
                    for h in range(HPC):
                        ps = up_ps.tile([P, 512], f32, tag="up")
                        for k in range(NKVM):
                            nc.tensor.matmul(
                                ps, wkvb[:, k, h * NOPE:(h + 1) * NOPE],
                                kv_lat[:, c, k, :], start=(k == 0),
                                stop=(k == NKVM - 1))
                        nc.vector.tensor_mul(
                            kt_nope[h][:, c * 512:(c + 1) * 512], ps,
                            a_kv_bc[:, c, :])

                def prescale(c):
                    for k in range(NKVM):
                        nc.vector.tensor_mul(kv_lat[:, c, k, :],
                                             kv_lat[:, c, k, :],
                                             a_kv_bc[:, c, :])

                def v_up(c):
                    for part in range(4):
                        sb = c * 4 + part
                        ps = up_ps.tile([P, HPC * V_DIM], f32, tag="up")
                        for k in range(NKVM):
                            nc.tensor.matmul(
                                ps, kv_lat[:, c, k, part * P:(part + 1) * P],
                                wkvb[:, k, HPC * NOPE:], start=(k == 0),
                                stop=(k == NKVM - 1))
                        nc.vector.tensor_copy(v_all[:, sb, :], ps)

                kt_up(0)
                kt_up(1)
                prescale(0)
                kt_up(2)
                prescale(1)
                v_up(0)
                kt_up(3)
                prescale(2)
                v_up(1)
                prescale(3)
                v_up(2)
                v_up(3)

                # ---- q up-proj (waits on AG_q1/2; q_lat streamed) ----
                for c in range(NCH):
                    ql = qlat_pool.tile([P, NQM, 512], bf, tag="ql",
                                        name="ql")
                    nc.sync.dma_start(
                        out=ql[:, 0:NQ1, :],
                        in_=g_q1[bass.ds(q1_base + c * Q1ROWS, Q1ROWS - 1),
                                 :].rearrange("(kt p) s -> p kt s", p=P))
                    nc.sync.dma_start(
                        out=ql[:, NQ1:, :],
                        in_=g_q2[bass.ds(q2_base + c * Q2ROWS, Q2ROWS), :]
                        .rearrange("(kt p) s -> p kt s", p=P))
                    for h in range(HPC):
                        ps = up_ps.tile([P, 512], f32, tag="up")
                        for k in range(NQM):
                            nc.tensor.matmul(
                                ps, wqb[:, k, h * P:(h + 1) * P],
                                ql[:, k, :], start=(k == 0),
                                stop=(k == NQM - 1))
                        nc.vector.tensor_mul(
                            qt_nope[h][:, c * 512:(c + 1) * 512], ps,
                            a_q_bc[:, c, :])
                    for hp in range(HPC // 2):
                        # both heads' rope columns are adjacent in wqb, so
                        # one 128-wide chain fills both 64-row halves
                        pcol = HPC * NOPE + 2 * hp * ROPE
                        ps = pe_ps.tile([P, 512], f32, tag="qp")
                        for k in range(NQM):
                            nc.tensor.matmul(
                                ps, wqb[:, k, pcol:pcol + 2 * ROPE],
                                ql[:, k, :], start=(k == 0),
                                stop=(k == NQM - 1))
                        pe_s = up.tile([P, 512], bf, tag="pes")
                        nc.vector.tensor_mul(pe_s, ps, a_q_bc[:, c, :])
                        xs = pe_ps.tile([P, 512], f32, tag="qpx")
                        nc.tensor.matmul(xs, perm_sb, pe_s)
                        dst = qt_pe[hp][:, c * 512:(c + 1) * 512]
                        nc.vector.tensor_mul(
                            dst, pe_s, cos_f_sb[:, c * 512:(c + 1) * 512])
                        t1 = up.tile([P, 512], bf, tag="pet")
                        nc.vector.tensor_mul(
                            t1, xs, sin_f_sb[:, c * 512:(c + 1) * 512])
                        nc.vector.tensor_add(dst, dst, t1)

            up_lat_cm.__exit__(None, None, None)

            # ========== attention + per-head A2As + interleaved wo ==========
            with (
                tc.tile_pool(name="at", bufs=8) as at,
                tc.tile_pool(name="at_ev", bufs=2) as at_ev,
                tc.tile_pool(name="at_ou", bufs=5) as at_ou,
                tc.tile_pool(name="at_rl", bufs=2) as at_rl,
                tc.tile_pool(name="wo_rhs", bufs=2) as wo_rhs,
                tc.tile_pool(name="wo_acc", bufs=1) as wo_acc,
                tc.tile_pool(name="wo_w", bufs=2) as wo_w,
                tc.tile_pool(name="wo_ev", bufs=3) as wo_ev,
                tc.tile_pool(name="st_ps", bufs=3, space="PSUM") as st_ps,
                tc.tile_pool(name="ot_ps", bufs=2, space="PSUM") as ot_ps,
                tc.tile_pool(name="l_ps", bufs=1, space="PSUM") as l_ps,
                tc.tile_pool(name="wo_ps", bufs=2, space="PSUM") as wo_ps,
            ):
                acc = wo_acc.tile([P, NKT, 512], f32)

                def attention_head(h):
                    pending = []

                    def flush(keep):
                        while len(pending) > keep:
                            pj, off, j, ot, lt, first, last = pending.pop(0)
                            nc.tensor.matmul(lt[:, off:], ones_b,
                                             pj[:, off:],
                                             start=first, stop=last)
                            nc.tensor.matmul(
                                ot[:, off:],
                                v_all[:, j, h * V_DIM:(h + 1) * V_DIM],
                                pj[:, off:], start=first, stop=last)

                    for qc in range(NCH):
                        nj = qc * 4 + 4
                        ot = ot_ps.tile([P, 512], f32, tag="ot", name="ot")
                        lt = l_ps.tile([1, 512], f32, tag="l", name="lt")
                        for j in range(nj):
                            d = j - qc * 4
                            off = max(0, d) * P
                            diag = d >= 0
                            st = st_ps.tile([P, 512], f32, tag="st",
                                            name="st")
                            if diag:
                                # causal mask prefilled into PSUM; score
                                # matmuls accumulate on top (keeps the DVE
                                # off the st->exp critical path)
                                nc.vector.tensor_copy(st[:, off:],
                                                      mask_sb[:, :512 - off])
                            nc.tensor.matmul(
                                st[:, off:],
                                kt_nope[h][:, j * P:(j + 1) * P],
                                qt_nope[h][:, qc * 512 + off:(qc + 1) * 512],
                                start=not diag, stop=False,
                                skip_group_check=diag)
                            lo = (h % 2) * ROPE
                            nc.tensor.matmul(
                                st[:, off:],
                                kpe_dup[lo:lo + ROPE, j // 4,
                                        (j % 4) * P:(j % 4 + 1) * P],
                                qt_pe[h // 2][lo:lo + ROPE,
                                              qc * 512 + off:(qc + 1) * 512],
                                start=False, stop=True,
                                skip_group_check=diag)
                            flush(3)
                            pj = at.tile([P, 512], bf, tag="p", name="pj")
                            nc.scalar.activation(
                                pj[:, off:], st[:, off:],
                                mybir.ActivationFunctionType.Exp)
                            pending.append((pj, off, j, ot, lt,
                                            j == 0, j == nj - 1))
                        flush(0)
                        # evict this qc while the next one computes
                        rl = at_rl.tile([1, 512], f32, tag="rl", name="rl")
                        nc.vector.reciprocal(rl, lt)
                        rlb = at_rl.tile([P, 512], f32, tag="rlb",
                                         name="rlb")
                        nc.gpsimd.partition_broadcast(rlb, rl)
                        ev = at_ev.tile([P, 512], bf, tag="oev", name="oev")
                        nc.vector.tensor_mul(ev, ot, rlb)
                        nc.sync.dma_start(
                            out=b_o[h][bass.ds(ob_row + qc * P, P), :],
                            in_=ev)
                    # shard (b*4+qc) -> rank (b, qc); received row block
                    # (b*4+k) = batch peer k's shard for our seq-cols =
                    # head 4k+h, matching woTr rows
                    nc.gpsimd.collective_compute(
                        "AllToAll", mybir.AluOpType.bypass,
                        replica_groups=rg8, ins=[b_o[h][:]],
                        outs=[o_t[h][:]])
                    # rhs for the wo pass of this head (gpsimd queue,
                    # already blocked on this A2A)
                    rhs = wo_rhs.tile([P, GROUP, 512], bf, tag="rhs",
                                      name="rhs")
                    for k in range(GROUP):
                        nc.gpsimd.dma_start(
                            out=rhs[:, k, :],
                            in_=o_t[h][bass.ds(ob_row + k * P, P), :])
                    # prefetch this head's wo weights (no deps)
                    wslab = wo_w.tile([P, GROUP, DIM], bf, tag="woslab",
                                      name="wslab")
                    nc.scalar.dma_start(
                        out=wslab,
                        in_=woTr[h * 512:(h + 1) * 512, :].rearrange(
                            "(kt p) m -> p kt m", p=P))
                    return rhs, wslab

                def wo_pass(h, rhs, wslab):
                    for m in range(NKT):
                        ps = wo_ps.tile([P, 512], f32, tag="wops",
                                        name="wops")
                        for k in range(GROUP):
                            nc.tensor.matmul(
                                ps, wslab[:, k, m * P:(m + 1) * P],
                                rhs[:, k, :], start=(k == 0),
                                stop=(k == GROUP - 1))
                        if h == 0:
                            nc.vector.tensor_copy(acc[:, m, :], ps)
                        elif h < HPC - 1:
                            nc.vector.tensor_add(acc[:, m, :], ps,
                                                 acc[:, m, :])
                        else:
                            ev = wo_ev.tile([P, 512], bf, tag="woev",
                                            name="woev")
                            nc.vector.tensor_add(ev, ps, acc[:, m, :])
                            # alternate queues so the final stores drain 2x
                            eng = nc.sync if m % 2 == 0 else nc.scalar
                            eng.dma_start(out=outT[m * P:(m + 1) * P, :],
                                          in_=ev)

                heads_rhs = {}
                for h in range(HPC):
                    heads_rhs[h] = attention_head(h)
                    if h >= 1:
                        wo_pass(h - 1, *heads_rhs[h - 1])
                wo_pass(HPC - 1, *heads_rhs[HPC - 1])

    nc.compile()
    return nc


def _prep_inputs(x, freqs_cos, freqs_sin, wq_a, q_norm_w, wq_b, wkv_a,
                 kv_norm_w, wkv_b, wo):
    x = np.asarray(x, np.float32)
    freqs_cos = np.asarray(freqs_cos, np.float32)
    freqs_sin = np.asarray(freqs_sin, np.float32)
    wq_a = np.asarray(wq_a, np.float32)
    q_norm_w = np.asarray(q_norm_w, np.float32)
    wq_b = np.asarray(wq_b, np.float32)
    wkv_a = np.asarray(wkv_a, np.float32)
    kv_norm_w = np.asarray(kv_norm_w, np.float32)
    wkv_b = np.asarray(wkv_b, np.float32)
    wo = np.asarray(wo, np.float32)

    # slab-contiguous down-proj weights: row (slab*128+p), col (kt*mw+m)
    # so each slab DMA is one 4KB-per-partition contiguous read
    def slabify(wT, nslab, mw):
        # wT: [DIM, M]; slab s covers out-dims [s*mw:(s+1)*mw]
        a = wT.reshape(NKT, P, nslab, mw)          # [kt, p, s, m]
        return np.ascontiguousarray(
            a.transpose(2, 1, 0, 3).reshape(nslab * P, NKT * mw))

    wqa_sl = slabify(wq_a.T, NQM, P).astype(BF16)
    wkva_sl = slabify(wkv_a.T[:, :KV_LORA], NKVM, P).astype(BF16)
    wkvr_sl = slabify(wkv_a.T[:, KV_LORA:], 1, ROPE).astype(BF16)

    wqb_eff = (wq_b * q_norm_w[None, :]) * SCALE
    wqb_eff = wqb_eff.reshape(N_HEADS, QK_HD, Q_LORA)
    wkvb_eff = wkv_b * kv_norm_w[None, :]
    wkvb_eff = wkvb_eff.reshape(N_HEADS, NOPE + V_DIM, KV_LORA)

    cosT = np.tile(np.repeat(freqs_cos.T, 2, axis=0), (2, 1))  # [128, S]
    sinT = np.tile(np.repeat(freqs_sin.T, 2, axis=0), (2, 1))

    perm64_ = np.zeros((ROPE, ROPE), np.float32)
    for i in range(ROPE // 2):
        perm64_[2 * i + 1, 2 * i] = -1.0  # out[2i]   = -x[2i+1]
        perm64_[2 * i, 2 * i + 1] = 1.0   # out[2i+1] =  x[2i]
    perm = np.zeros((P, P), np.float32)
    perm[:ROPE, :ROPE] = perm64_
    perm[ROPE:, ROPE:] = perm64_
    r = np.arange(P)
    trimask = np.zeros((P, 512), np.float32)
    trimask[:, :P] = np.where(r[:, None] <= r[None, :], 0.0, -1e30)

    # wo.T rows regrouped so pass h contracts head j*4+h for j=0..3:
    # woTr rows [h*512 + j*128 : ...] = wo.T rows of head j*4+h
    woT4 = wo.T.reshape(N_HEADS // 4, 4, V_DIM, DIM)  # [j, h, 128, D]
    woTr = np.ascontiguousarray(
        woT4.transpose(1, 0, 2, 3).reshape(N_HEADS * V_DIM, DIM)).astype(BF16)

    in_maps = []
    for c in range(NCORES):
        b, g = c // GROUP, c % GROUP
        heads = slice(g * HPC, (g + 1) * HPC)
        xTc = np.ascontiguousarray(
            x[b].T[:, g * SSH:(g + 1) * SSH]).astype(BF16)
        wqbT = np.concatenate(
            [wqb_eff[heads, :NOPE].reshape(HPC * NOPE, Q_LORA),
             wqb_eff[heads, NOPE:].reshape(HPC * ROPE, Q_LORA)],
            axis=0).T
        wkvbT = np.concatenate(
            [wkvb_eff[heads, :NOPE].reshape(HPC * NOPE, KV_LORA),
             wkvb_eff[heads, NOPE:].reshape(HPC * V_DIM, KV_LORA)],
            axis=0).T
        in_maps.append({
            "xT": xTc,
            "wqa_sl": wqa_sl,
            "wkva_sl": wkva_sl,
            "wkvr_sl": wkvr_sl,
            "wqbT": np.ascontiguousarray(wqbT).astype(BF16),
            "wkvbT": np.ascontiguousarray(wkvbT).astype(BF16),
            "woTr": woTr,
            "cos_sh": np.ascontiguousarray(
                cosT[:, g * SSH:(g + 1) * SSH]).astype(BF16),
            "sin_sh": np.ascontiguousarray(
                sinT[:, g * SSH:(g + 1) * SSH]).astype(BF16),
            "cos_full": np.ascontiguousarray(cosT).astype(BF16),
            "sin_full": np.ascontiguousarray(sinT).astype(BF16),
            "perm64": perm.astype(BF16),
            "trimask": trimask,
            "cfg": np.array([[b * GROUP * KVROWS, b * GROUP * Q1ROWS,
                              b * GROUP * Q2ROWS, b * GROUP * V_DIM, 0]],
                            np.int32),
        })
    return in_maps


def _run(inputs, trace=False, **kw):
    if "nc" not in _cache:
        _cache["nc"] = _build()
    nc = _cache["nc"]
    in_maps = _prep_inputs(**inputs)
    res = run_bass_kernel_spmd(nc, in_maps, core_ids=list(range(NCORES)),
                               trace=trace, **kw)
    out = np.empty((B, S, DIM), np.float32)
    for c in range(NCORES):
        b, g = c // GROUP, c % GROUP
        out[b, g * SSH:(g + 1) * SSH, :] = \
            res.results[c]["out"].astype(np.float32).T
    return out, res


def kernel(**inputs):
    out, _ = _run(inputs)
    return out
